# revision 4
# baseline (speedup 1.0000x reference)
"""Trainium2 Bass kernel for nn_KeyHeatModule (8-core data parallel).

Strategy: shard batch (4096) across 8 cores (512 each). Host pre-transposes x
so each core streams contiguous [feat, batch] tiles; one fused weight matrix
W_all (8330 x 126) is replicated. Per core: fp32 matmul accumulates
vec.T = W_all.T @ x_shard.T in PSUM per 128-sample pass, PE-transposes back to
batch-major, then DVE/ACT compute softmax/exp, keypoint range masks, segmented
cumsum heatmap math, per-group max normalization. Output (512, 441) per core is
gathered and split into the 5 group tensors on host.
"""
import sys
for _p in ("/opt/trn_rl_repo", "/root/.axon_site/_ro/trn_rl_repo"):
    if _p not in sys.path:
        sys.path.insert(0, _p)

import numpy as np

N_CORES = 8
BS = 4096
F = 8330          # in features
M = 126           # total linear out cols (28*4 + 14)
WD = 7
SCALE = 0.3
BPC = BS // N_CORES          # 512 samples per core
import os as _os
N_PASS = int(_os.environ.get("KH_N_PASS", "4"))   # batch passes per core
Bp = BPC // N_PASS
NCH = 9                      # total channels across groups (2,2,2,2,1)
KEY_GATHER = [0, 1, 3, 4, 7, 8, 9, 10, 5, 6, 11, 12, 11, 12, 13, 14, 15, 16]
GROUP_SIZES = [4, 4, 4, 4, 2]    # gathered keypoint count per group
KT_FULL = F // 128           # 65 full k-tiles
K_TAIL = F - KT_FULL * 128   # 10
KBW = 8                      # W k-tiles per DMA batch
KBX = 16                     # x k-tiles per DMA batch

_cached = {}


def _consts_np():
    """Host-built constant tile (128, 205)."""
    c = np.zeros((128, 205), np.float32)
    seg = np.ones(63, np.float32)
    seg[0::7] = 0.0                       # reset at w==0 of each 7-block
    c[:, 0:63] = seg[None, :]
    c[:, 63:70] = (np.arange(7) + 2).astype(np.float32)[None, :]   # iota2
    cap = np.arange(7).astype(np.float32)
    cap[6] = 1e9                          # w=6 never kept by (w < hi<=6)
    c[:, 70:77] = cap[None, :]
    c[:, 77:205] = np.eye(128, dtype=np.float32)
    return c


def build_program(n_reps=1):
    import concourse.bacc as bacc
    import concourse.tile as tile
    from concourse import mybir

    f32 = mybir.dt.float32
    A = mybir.AluOpType
    ACT = mybir.ActivationFunctionType
    AX = mybir.AxisListType

    nc = bacc.Bacc(num_devices=N_CORES)
    xT_d = nc.dram_tensor("xT", (N_PASS * F, Bp), f32, kind="ExternalInput")
    w_d = nc.dram_tensor("w", (F, M), f32, kind="ExternalInput")
    b_d = nc.dram_tensor("b", (M,), f32, kind="ExternalInput")
    kp_d = nc.dram_tensor("kp", (BPC, 36), f32, kind="ExternalInput")
    c_d = nc.dram_tensor("consts", (128, 205), f32, kind="ExternalInput")
    out_d = nc.dram_tensor("out", (BPC, 441), f32, kind="ExternalOutput")

    def bc(ap, shape):
        return ap.to_broadcast(tuple(shape))

    with tile.TileContext(nc) as tc:
        import contextlib
        with contextlib.ExitStack() as ctx:
            singles = ctx.enter_context(tc.tile_pool(name="singles", bufs=1))
            xpool = ctx.enter_context(tc.tile_pool(name="xpool", bufs=3))
            pspool = ctx.enter_context(tc.tile_pool(name="ps", bufs=2, space="PSUM"))
            work = ctx.enter_context(tc.tile_pool(name="work", bufs=2))

            # ---- constants / weights (once) ----
            CT = singles.tile([128, 205], f32)
            nc.sync.dma_start(CT[:], c_d[:, :])
            SEG = CT[:, 0:63]
            IOTA2 = CT[:, 63:70]
            IOCAP = CT[:, 70:77]
            IDENT = CT[:, 77:205]

            BT = singles.tile([128, 1], f32)
            nc.sync.dma_start(BT[:126, :], b_d[:].rearrange("(p o) -> p o", o=1))

            wtiles = {}          # ktile idx -> AP [128 or 10, 126]
            wb_list = []
            for i, k0 in enumerate(range(0, KT_FULL, KBW)):
                nk = min(KBW, KT_FULL - k0)
                wt = singles.tile([128, nk, M], f32, tag=f"wb{i}")
                nc.sync.dma_start(
                    wt[:],
                    w_d[k0 * 128:(k0 + nk) * 128, :].rearrange(
                        "(s p) m -> p s m", p=128))
                wb_list.append(wt)
                for s in range(nk):
                    wtiles[k0 + s] = wt[:, s, :]
            WTAIL = singles.tile([128, M], f32, tag="wtail")
            nc.sync.dma_start(WTAIL[:K_TAIL, :], w_d[KT_FULL * 128:F, :])

            for rep in range(n_reps):
                for p in range(N_PASS):
                    # ---- x DMA (contiguous ~1MB batches) + matmul ----
                    xbs = []
                    for k0 in range(0, KT_FULL, KBX):
                        nk = min(KBX, KT_FULL - k0)
                        xb = xpool.tile([128, nk, Bp], f32, tag="xb")
                        r0 = p * F + k0 * 128
                        nc.sync.dma_start(
                            xb[:, :nk, :],
                            xT_d[r0:r0 + nk * 128, :].rearrange(
                                "(s q) n -> q s n", q=128))
                        xbs.append((k0, nk, xb))
                    xtail = xpool.tile([128, Bp], f32, tag="xtail")
                    nc.sync.dma_start(
                        xtail[:K_TAIL, :],
                        xT_d[p * F + KT_FULL * 128:p * F + F, :])

                    ps = pspool.tile([M, Bp], f32, tag="ps")
                    first = True
                    for (k0, nk, xb) in xbs:
                        for s in range(nk):
                            nc.tensor.matmul(
                                ps[:], wtiles[k0 + s], xb[:, s, :],
                                start=first, stop=False)
                            first = False
                    nc.tensor.matmul(ps[:], WTAIL[:K_TAIL, :],
                                     xtail[:K_TAIL, :], start=False, stop=True)

                    for cc in range(Bp // 128):
                        c = p * (Bp // 128) + cc      # global chunk 0..3
                        # ---- bias add + transpose to batch-major ----
                        VEC = work.tile([128, 128], f32, tag="vec")
                        nc.scalar.activation(
                            VEC[:M, :], ps[:, cc * 128:(cc + 1) * 128],
                            ACT.Identity, bias=BT[:M, :], scale=1.0)
                        TP = pspool.tile([128, M], f32, tag="tp")
                        nc.tensor.transpose(TP[:], VEC[:M, :], IDENT[:M, :M])
                        VB = work.tile([128, M], f32, tag="vb")
                        nc.scalar.copy(VB[:], TP[:])

                        # ---- keypoint tile + group ranges ----
                        KPT = work.tile([128, 36], f32, tag="kpt")
                        nc.sync.dma_start(KPT[:], kp_d[c * 128:(c + 1) * 128, :])
                        MMs = work.tile([128, 20], f32, tag="mms")
                        for j, (off, op) in enumerate(
                                [(0, A.min), (0, A.max), (18, A.min), (18, A.max)]):
                            nc.vector.tensor_reduce(
                                MMs[:, j * 5:j * 5 + 4],
                                KPT[:, off:off + 16].rearrange(
                                    "p (g k) -> p g k", k=4),
                                AX.X, op)
                            nc.vector.tensor_reduce(
                                MMs[:, j * 5 + 4:j * 5 + 5],
                                KPT[:, off + 16:off + 18], AX.X, op)
                        # channel-level stats CS [128, 36]
                        CS = work.tile([128, 36], f32, tag="cs")
                        for j in range(4):
                            nc.gpsimd.tensor_copy(
                                CS[:, j * 9:j * 9 + 8].rearrange(
                                    "p (g r) -> p g r", r=2),
                                bc(MMs[:, j * 5:j * 5 + 4].rearrange(
                                    "p (g o) -> p g o", o=1), (128, 4, 2)))
                            nc.gpsimd.tensor_copy(
                                CS[:, j * 9 + 8:j * 9 + 9],
                                MMs[:, j * 5 + 4:j * 5 + 5])

                        # ---- keeps [128, 63] each ----
                        KEEP = work.tile([128, 4, NCH, WD], f32, tag="keep")
                        i2b = bc(IOTA2.rearrange("p (o w) -> p o w", o=1),
                                 (128, NCH, WD))
                        icb = bc(IOCAP.rearrange("p (o w) -> p o w", o=1),
                                 (128, NCH, WD))
                        for j, (cso, one_is_iota) in enumerate(
                                [(0, True), (9, False), (18, True), (27, False)]):
                            csb = bc(CS[:, cso:cso + 9].rearrange(
                                "p (c o) -> p c o", o=1), (128, NCH, WD))
                            if one_is_iota:   # keep1 = iota2 > fmin
                                nc.vector.tensor_tensor(
                                    KEEP[:, j, :, :], i2b, csb, A.is_gt)
                            else:             # keep2 = fmax >= iota_cap
                                nc.vector.tensor_tensor(
                                    KEEP[:, j, :, :], csb, icb, A.is_ge)

                        # ---- softmax-pair + exp ----
                        VB3 = VB.rearrange("p (c s w) -> p c s w", s=2, w=WD)
                        D = work.tile([128, NCH, WD], f32, tag="d")
                        nc.vector.tensor_tensor(
                            D[:], VB3[:, :, 0, :], VB3[:, :, 1, :], A.subtract)
                        V0 = work.tile([128, NCH, WD], f32, tag="v0")
                        V1 = work.tile([128, NCH, WD], f32, tag="v1")
                        S0 = work.tile([128, NCH, WD], f32, tag="s0")
                        S1 = work.tile([128, NCH, WD], f32, tag="s1")
                        nc.scalar.activation(S0[:], D[:], ACT.Sigmoid)
                        nc.scalar.activation(S1[:], D[:], ACT.Sigmoid, scale=-1.0)
                        nc.scalar.activation(V0[:], S0[:], ACT.Exp)
                        nc.scalar.activation(V1[:], S1[:], ACT.Exp)

                        # ---- masked segmented cumsum -> thresholds ----
                        AM = work.tile([128, 4, NCH, WD], f32, tag="am")
                        for j, V in enumerate([V0, V0, V1, V1]):
                            nc.vector.tensor_mul(AM[:, j, :, :], V[:],
                                                 KEEP[:, j, :, :])
                        C = work.tile([128, 4, NCH, WD], f32, tag="cum")
                        for j in range(4):
                            nc.vector.tensor_tensor_scan(
                                C[:, j, :, :].rearrange("p c w -> p (c w)"),
                                SEG,
                                AM[:, j, :, :].rearrange("p c w -> p (c w)"),
                                0.0, A.mult, A.add)
                        R = work.tile([128, 4, NCH], f32, tag="recip")
                        for j in range(4):
                            nc.vector.reciprocal(
                                R[:, j, :].unsqueeze(2),
                                C[:, j, :, 6:7])
                        TH = work.tile([128, 4, NCH, WD], f32, tag="th")
                        T2 = work.tile([128, 2, NCH, WD], f32, tag="t2")
                        for j in range(4):
                            rb = bc(R[:, j, :].unsqueeze(2),
                                    (128, NCH, WD))
                            if j % 2 == 0:    # c1 = cum/S ; thresh
                                nc.vector.tensor_mul(TH[:, j, :, :],
                                                     C[:, j, :, :], rb)
                                nc.vector.scalar_tensor_tensor(
                                    TH[:, j, :, :], TH[:, j, :, :], SCALE,
                                    TH[:, j, :, :], A.is_ge, A.mult)
                            else:             # c2 = 1 - cum/S ; thresh
                                t2 = T2[:, j // 2, :, :]
                                nc.vector.tensor_mul(t2, C[:, j, :, :], rb)
                                nc.vector.tensor_scalar(
                                    t2, t2, -1.0, 1.0, A.mult, A.add)
                                nc.vector.scalar_tensor_tensor(
                                    TH[:, j, :, :], t2, SCALE, t2,
                                    A.is_ge, A.mult)
                        XM = work.tile([128, NCH, WD], f32, tag="xm")
                        YM = work.tile([128, NCH, WD], f32, tag="ym")
                        nc.vector.tensor_mul(XM[:], TH[:, 0, :, :], TH[:, 1, :, :])
                        nc.vector.tensor_mul(YM[:], TH[:, 2, :, :], TH[:, 3, :, :])

                        # ---- outer product + per-group max normalize ----
                        MO = work.tile([128, NCH, WD, WD], f32, tag="mo")
                        nc.vector.tensor_mul(
                            MO[:],
                            bc(YM[:].unsqueeze(3),
                               (128, NCH, WD, WD)),
                            bc(XM[:].unsqueeze(2),
                               (128, NCH, WD, WD)))
                        CHM = work.tile([128, NCH], f32, tag="chm")
                        nc.vector.tensor_reduce(
                            CHM[:], MO.rearrange("p c y x -> p c (y x)"),
                            AX.X, A.max)
                        GM = work.tile([128, 8], f32, tag="gm")
                        CHM3 = CHM[:, 0:8].rearrange("p (g r) -> p g r", r=2)
                        nc.vector.tensor_tensor(
                            GM[:, 0:4].rearrange("p (g o) -> p g o", o=1),
                            CHM3[:, :, 0:1], CHM3[:, :, 1:2], A.max)
                        nc.gpsimd.tensor_copy(GM[:, 4:5], CHM[:, 8:9])
                        nc.vector.tensor_scalar(GM[:, 5:6], GM[:, 4:5], 1e-7,
                                                None, A.add)
                        RM = work.tile([128, 9], f32, tag="rm")
                        nc.vector.tensor_scalar(GM[:, 0:4], GM[:, 0:4], 1e-7,
                                                None, A.add)
                        nc.vector.reciprocal(RM[:, 0:4], GM[:, 0:4])
                        nc.vector.reciprocal(RM[:, 4:5], GM[:, 5:6])
                        RC = work.tile([128, NCH], f32, tag="rc")
                        nc.gpsimd.tensor_copy(
                            RC[:, 0:8].rearrange("p (g r) -> p g r", r=2),
                            bc(RM[:, 0:4].rearrange("p (g o) -> p g o", o=1),
                               (128, 4, 2)))
                        nc.gpsimd.tensor_copy(RC[:, 8:9], RM[:, 4:5])
                        MN = work.tile([128, NCH, WD * WD], f32, tag="mn")
                        nc.vector.tensor_mul(
                            MN[:], MO.rearrange("p c y x -> p c (y x)"),
                            bc(RC[:].unsqueeze(2), (128, NCH, WD * WD)))
                        nc.sync.dma_start(out_d[c * 128:(c + 1) * 128, :],
                                          MN.rearrange("p c q -> p (c q)"))
    nc.compile()
    return nc


def _prep_inputs(x, keypoint, W_all, b_all):
    """Host-side sharding/layout. Returns per-core in_maps."""
    xT = np.ascontiguousarray(x.T)            # (F, BS)
    kp7 = keypoint[:, KEY_GATHER, :].astype(np.float32) * np.float32(7.0)
    kpg = np.concatenate([kp7[:, :, 0], kp7[:, :, 1]], axis=1)  # (BS, 36)
    kpg = np.ascontiguousarray(kpg)
    consts = _consts_np()
    in_maps = []
    for core in range(N_CORES):
        b0 = core * BPC
        sh = xT[:, b0:b0 + BPC]               # (F, 512)
        # (N_PASS*F, Bp): pass p = cols p*Bp..(p+1)*Bp
        xs = np.concatenate(
            [np.ascontiguousarray(sh[:, p * Bp:(p + 1) * Bp])
             for p in range(N_PASS)], axis=0)
        in_maps.append({
            "xT": xs,
            "w": W_all,
            "b": b_all,
            "kp": np.ascontiguousarray(kpg[b0:b0 + BPC]),
            "consts": consts,
        })
    return in_maps


def _split_output(full):
    """(4096, 441) -> tuple of 5 group tensors."""
    outs = []
    off = 0
    for g in (2, 2, 2, 2, 1):
        outs.append(full[:, off:off + g * 49].reshape(BS, g, 1, WD, WD))
        off += g * 49
    return tuple(outs)


def kernel(**inputs):
    from concourse.bass_utils import run_bass_kernel_spmd

    W_all = np.concatenate(
        [inputs["W_head"], inputs["W_arm"], inputs["W_upper"],
         inputs["W_lower"], inputs["W_foot"]], axis=1).astype(np.float32)
    W_all = np.ascontiguousarray(W_all)
    b_all = np.concatenate(
        [inputs["b_head"], inputs["b_arm"], inputs["b_upper"],
         inputs["b_lower"], inputs["b_foot"]]).astype(np.float32)
    in_maps = _prep_inputs(np.asarray(inputs["x"], np.float32),
                           np.asarray(inputs["keypoint"], np.float32),
                           W_all, b_all)
    if "nc" not in _cached:
        _cached["nc"] = build_program(1)
    res = run_bass_kernel_spmd(_cached["nc"], in_maps,
                               core_ids=list(range(N_CORES)))
    full = np.concatenate([res.results[c]["out"] for c in range(N_CORES)],
                          axis=0)
    return _split_output(full)


# revision 5
# speedup vs baseline: 1.0930x; 1.0930x over previous
"""Trainium2 Bass kernel for nn_KeyHeatModule (8-core data parallel).

Strategy: shard batch (4096) across 8 cores (512 each). Host pre-transposes x
so each core streams contiguous [feat, batch] tiles; one fused weight matrix
W_all (8330 x 126) is replicated. Per core: fp32 matmul accumulates
vec.T = W_all.T @ x_shard.T in PSUM per 128-sample pass, PE-transposes back to
batch-major, then DVE/ACT compute softmax/exp, keypoint range masks, segmented
cumsum heatmap math, per-group max normalization. Output (512, 441) per core is
gathered and split into the 5 group tensors on host.
"""
import sys
for _p in ("/opt/trn_rl_repo", "/root/.axon_site/_ro/trn_rl_repo"):
    if _p not in sys.path:
        sys.path.insert(0, _p)

import numpy as np

N_CORES = 8
BS = 4096
F = 8330          # in features
M = 126           # total linear out cols (28*4 + 14)
WD = 7
SCALE = 0.3
BPC = BS // N_CORES          # 512 samples per core
import os as _os
N_PASS = int(_os.environ.get("KH_N_PASS", "4"))   # batch passes per core
Bp = BPC // N_PASS
NCH = 9                      # total channels across groups (2,2,2,2,1)
KEY_GATHER = [0, 1, 3, 4, 7, 8, 9, 10, 5, 6, 11, 12, 11, 12, 13, 14, 15, 16]
GROUP_SIZES = [4, 4, 4, 4, 2]    # gathered keypoint count per group
KT_FULL = F // 128           # 65 full k-tiles
K_TAIL = F - KT_FULL * 128   # 10
KBW = 8                      # W k-tiles per DMA batch
KBX = int(_os.environ.get("KH_KBX", "16"))   # x k-tiles per DMA batch
XBUFS = int(_os.environ.get("KH_XBUFS", "3"))
ABLATE = _os.environ.get("KH_ABLATE", "full")  # full | mm | dma

_cached = {}


def _consts_np():
    """Host-built constant tile (128, 205)."""
    c = np.zeros((128, 205), np.float32)
    seg = np.ones(63, np.float32)
    seg[0::7] = 0.0                       # reset at w==0 of each 7-block
    c[:, 0:63] = seg[None, :]
    c[:, 63:70] = (np.arange(7) + 2).astype(np.float32)[None, :]   # iota2
    cap = np.arange(7).astype(np.float32)
    cap[6] = 1e9                          # w=6 never kept by (w < hi<=6)
    c[:, 70:77] = cap[None, :]
    c[:, 77:205] = np.eye(128, dtype=np.float32)
    return c


def build_program(n_reps=1):
    import concourse.bacc as bacc
    import concourse.tile as tile
    from concourse import mybir

    f32 = mybir.dt.float32
    A = mybir.AluOpType
    ACT = mybir.ActivationFunctionType
    AX = mybir.AxisListType

    nc = bacc.Bacc(num_devices=N_CORES)
    xT_d = nc.dram_tensor("xT", (N_PASS * F, Bp), f32, kind="ExternalInput")
    w_d = nc.dram_tensor("w", (F, M), f32, kind="ExternalInput")
    b_d = nc.dram_tensor("b", (M,), f32, kind="ExternalInput")
    kp_d = nc.dram_tensor("kp", (BPC, 36), f32, kind="ExternalInput")
    c_d = nc.dram_tensor("consts", (128, 205), f32, kind="ExternalInput")
    out_d = nc.dram_tensor("out", (BPC, 441), f32, kind="ExternalOutput")

    def bc(ap, shape):
        return ap.to_broadcast(tuple(shape))

    with tile.TileContext(nc) as tc:
        import contextlib
        with contextlib.ExitStack() as ctx:
            singles = ctx.enter_context(tc.tile_pool(name="singles", bufs=1))
            xpool = ctx.enter_context(tc.tile_pool(name="xpool", bufs=XBUFS))
            pspool = ctx.enter_context(tc.tile_pool(name="ps", bufs=2, space="PSUM"))
            work = ctx.enter_context(tc.tile_pool(name="work", bufs=2))

            # ---- constants / weights (once) ----
            CT = singles.tile([128, 205], f32)
            nc.sync.dma_start(CT[:], c_d[:, :])
            SEG = CT[:, 0:63]
            IOTA2 = CT[:, 63:70]
            IOCAP = CT[:, 70:77]
            IDENT = CT[:, 77:205]

            BT = singles.tile([128, 1], f32)
            nc.sync.dma_start(BT[:126, :], b_d[:].rearrange("(p o) -> p o", o=1))

            wtiles = {}          # ktile idx -> AP [128 or 10, 126]
            wb_list = []
            for i, k0 in enumerate(range(0, KT_FULL, KBW)):
                nk = min(KBW, KT_FULL - k0)
                wt = singles.tile([128, nk, M], f32, tag=f"wb{i}")
                nc.sync.dma_start(
                    wt[:],
                    w_d[k0 * 128:(k0 + nk) * 128, :].rearrange(
                        "(s p) m -> p s m", p=128))
                wb_list.append(wt)
                for s in range(nk):
                    wtiles[k0 + s] = wt[:, s, :]
            WTAIL = singles.tile([128, M], f32, tag="wtail")
            nc.sync.dma_start(WTAIL[:K_TAIL, :], w_d[KT_FULL * 128:F, :])

            for rep in range(n_reps):
                for p in range(N_PASS):
                    # ---- x DMA (contiguous ~1MB batches) + matmul ----
                    xbs = []
                    for k0 in range(0, KT_FULL, KBX):
                        nk = min(KBX, KT_FULL - k0)
                        xb = xpool.tile([128, nk, Bp], f32, tag="xb")
                        r0 = p * F + k0 * 128
                        nc.sync.dma_start(
                            xb[:, :nk, :],
                            xT_d[r0:r0 + nk * 128, :].rearrange(
                                "(s q) n -> q s n", q=128))
                        xbs.append((k0, nk, xb))
                    xtail = xpool.tile([128, Bp], f32, tag="xtail")
                    nc.sync.dma_start(
                        xtail[:K_TAIL, :],
                        xT_d[p * F + KT_FULL * 128:p * F + F, :])

                    if ABLATE == "dma":
                        tch = work.tile([128, Bp], f32, tag="touch")
                        nc.vector.tensor_copy(tch[:], xbs[-1][2][:, 0, :])
                        nc.sync.dma_start(
                            out_d[p * Bp:p * Bp + 128, 0:Bp // 4],
                            tch[:, 0:Bp // 4])
                        continue
                    ps = pspool.tile([M, Bp], f32, tag="ps")
                    first = True
                    for (k0, nk, xb) in xbs:
                        for s in range(nk):
                            nc.tensor.matmul(
                                ps[:], wtiles[k0 + s], xb[:, s, :],
                                start=first, stop=False)
                            first = False
                    nc.tensor.matmul(ps[:], WTAIL[:K_TAIL, :],
                                     xtail[:K_TAIL, :], start=False, stop=True)

                    for cc in range(Bp // 128):
                        c = p * (Bp // 128) + cc      # global chunk 0..3
                        # ---- bias add + transpose to batch-major ----
                        VEC = work.tile([128, 128], f32, tag="vec")
                        nc.scalar.activation(
                            VEC[:M, :], ps[:, cc * 128:(cc + 1) * 128],
                            ACT.Identity, bias=BT[:M, :], scale=1.0)
                        TP = pspool.tile([128, M], f32, tag="tp")
                        nc.tensor.transpose(TP[:], VEC[:M, :], IDENT[:M, :M])
                        VB = work.tile([128, M], f32, tag="vb")
                        nc.scalar.copy(VB[:], TP[:])
                        if ABLATE == "mm":
                            nc.sync.dma_start(
                                out_d[c * 128:(c + 1) * 128, 0:M], VB[:])
                            continue

                        # ---- keypoint tile + group ranges ----
                        KPT = work.tile([128, 36], f32, tag="kpt")
                        nc.sync.dma_start(KPT[:], kp_d[c * 128:(c + 1) * 128, :])
                        MMs = work.tile([128, 20], f32, tag="mms")
                        for j, (off, op) in enumerate(
                                [(0, A.min), (0, A.max), (18, A.min), (18, A.max)]):
                            nc.vector.tensor_reduce(
                                MMs[:, j * 5:j * 5 + 4],
                                KPT[:, off:off + 16].rearrange(
                                    "p (g k) -> p g k", k=4),
                                AX.X, op)
                            nc.vector.tensor_reduce(
                                MMs[:, j * 5 + 4:j * 5 + 5],
                                KPT[:, off + 16:off + 18], AX.X, op)
                        # channel-level stats CS [128, 36]
                        CS = work.tile([128, 36], f32, tag="cs")
                        for j in range(4):
                            nc.gpsimd.tensor_copy(
                                CS[:, j * 9:j * 9 + 8].rearrange(
                                    "p (g r) -> p g r", r=2),
                                bc(MMs[:, j * 5:j * 5 + 4].rearrange(
                                    "p (g o) -> p g o", o=1), (128, 4, 2)))
                            nc.gpsimd.tensor_copy(
                                CS[:, j * 9 + 8:j * 9 + 9],
                                MMs[:, j * 5 + 4:j * 5 + 5])

                        # ---- keeps [128, 63] each ----
                        KEEP = work.tile([128, 4, NCH, WD], f32, tag="keep")
                        i2b = bc(IOTA2.rearrange("p (o w) -> p o w", o=1),
                                 (128, NCH, WD))
                        icb = bc(IOCAP.rearrange("p (o w) -> p o w", o=1),
                                 (128, NCH, WD))
                        for j, (cso, one_is_iota) in enumerate(
                                [(0, True), (9, False), (18, True), (27, False)]):
                            csb = bc(CS[:, cso:cso + 9].rearrange(
                                "p (c o) -> p c o", o=1), (128, NCH, WD))
                            if one_is_iota:   # keep1 = iota2 > fmin
                                nc.vector.tensor_tensor(
                                    KEEP[:, j, :, :], i2b, csb, A.is_gt)
                            else:             # keep2 = fmax >= iota_cap
                                nc.vector.tensor_tensor(
                                    KEEP[:, j, :, :], csb, icb, A.is_ge)

                        # ---- softmax-pair + exp ----
                        VB3 = VB.rearrange("p (c s w) -> p c s w", s=2, w=WD)
                        D = work.tile([128, NCH, WD], f32, tag="d")
                        nc.vector.tensor_tensor(
                            D[:], VB3[:, :, 0, :], VB3[:, :, 1, :], A.subtract)
                        V0 = work.tile([128, NCH, WD], f32, tag="v0")
                        V1 = work.tile([128, NCH, WD], f32, tag="v1")
                        S0 = work.tile([128, NCH, WD], f32, tag="s0")
                        S1 = work.tile([128, NCH, WD], f32, tag="s1")
                        nc.scalar.activation(S0[:], D[:], ACT.Sigmoid)
                        nc.scalar.activation(S1[:], D[:], ACT.Sigmoid, scale=-1.0)
                        nc.scalar.activation(V0[:], S0[:], ACT.Exp)
                        nc.scalar.activation(V1[:], S1[:], ACT.Exp)

                        # ---- masked segmented cumsum -> thresholds ----
                        AM = work.tile([128, 4, NCH, WD], f32, tag="am")
                        for j, V in enumerate([V0, V0, V1, V1]):
                            nc.vector.tensor_mul(AM[:, j, :, :], V[:],
                                                 KEEP[:, j, :, :])
                        C = work.tile([128, 4, NCH, WD], f32, tag="cum")
                        for j in range(4):
                            nc.vector.tensor_tensor_scan(
                                C[:, j, :, :].rearrange("p c w -> p (c w)"),
                                SEG,
                                AM[:, j, :, :].rearrange("p c w -> p (c w)"),
                                0.0, A.mult, A.add)
                        R = work.tile([128, 4, NCH], f32, tag="recip")
                        for j in range(4):
                            nc.vector.reciprocal(
                                R[:, j, :].unsqueeze(2),
                                C[:, j, :, 6:7])
                        TH = work.tile([128, 4, NCH, WD], f32, tag="th")
                        T2 = work.tile([128, 2, NCH, WD], f32, tag="t2")
                        for j in range(4):
                            rb = bc(R[:, j, :].unsqueeze(2),
                                    (128, NCH, WD))
                            if j % 2 == 0:    # c1 = cum/S ; thresh
                                nc.vector.tensor_mul(TH[:, j, :, :],
                                                     C[:, j, :, :], rb)
                                nc.vector.scalar_tensor_tensor(
                                    TH[:, j, :, :], TH[:, j, :, :], SCALE,
                                    TH[:, j, :, :], A.is_ge, A.mult)
                            else:             # c2 = 1 - cum/S ; thresh
                                t2 = T2[:, j // 2, :, :]
                                nc.vector.tensor_mul(t2, C[:, j, :, :], rb)
                                nc.vector.tensor_scalar(
                                    t2, t2, -1.0, 1.0, A.mult, A.add)
                                nc.vector.scalar_tensor_tensor(
                                    TH[:, j, :, :], t2, SCALE, t2,
                                    A.is_ge, A.mult)
                        XM = work.tile([128, NCH, WD], f32, tag="xm")
                        YM = work.tile([128, NCH, WD], f32, tag="ym")
                        nc.vector.tensor_mul(XM[:], TH[:, 0, :, :], TH[:, 1, :, :])
                        nc.vector.tensor_mul(YM[:], TH[:, 2, :, :], TH[:, 3, :, :])

                        # ---- outer product + per-group max normalize ----
                        MO = work.tile([128, NCH, WD, WD], f32, tag="mo")
                        nc.vector.tensor_mul(
                            MO[:],
                            bc(YM[:].unsqueeze(3),
                               (128, NCH, WD, WD)),
                            bc(XM[:].unsqueeze(2),
                               (128, NCH, WD, WD)))
                        CHM = work.tile([128, NCH], f32, tag="chm")
                        nc.vector.tensor_reduce(
                            CHM[:], MO.rearrange("p c y x -> p c (y x)"),
                            AX.X, A.max)
                        GM = work.tile([128, 8], f32, tag="gm")
                        CHM3 = CHM[:, 0:8].rearrange("p (g r) -> p g r", r=2)
                        nc.vector.tensor_tensor(
                            GM[:, 0:4].rearrange("p (g o) -> p g o", o=1),
                            CHM3[:, :, 0:1], CHM3[:, :, 1:2], A.max)
                        nc.gpsimd.tensor_copy(GM[:, 4:5], CHM[:, 8:9])
                        nc.vector.tensor_scalar(GM[:, 5:6], GM[:, 4:5], 1e-7,
                                                None, A.add)
                        RM = work.tile([128, 9], f32, tag="rm")
                        nc.vector.tensor_scalar(GM[:, 0:4], GM[:, 0:4], 1e-7,
                                                None, A.add)
                        nc.vector.reciprocal(RM[:, 0:4], GM[:, 0:4])
                        nc.vector.reciprocal(RM[:, 4:5], GM[:, 5:6])
                        RC = work.tile([128, NCH], f32, tag="rc")
                        nc.gpsimd.tensor_copy(
                            RC[:, 0:8].rearrange("p (g r) -> p g r", r=2),
                            bc(RM[:, 0:4].rearrange("p (g o) -> p g o", o=1),
                               (128, 4, 2)))
                        nc.gpsimd.tensor_copy(RC[:, 8:9], RM[:, 4:5])
                        MN = work.tile([128, NCH, WD * WD], f32, tag="mn")
                        nc.vector.tensor_mul(
                            MN[:], MO.rearrange("p c y x -> p c (y x)"),
                            bc(RC[:].unsqueeze(2), (128, NCH, WD * WD)))
                        nc.sync.dma_start(out_d[c * 128:(c + 1) * 128, :],
                                          MN.rearrange("p c q -> p (c q)"))
    nc.compile()
    return nc


def _prep_inputs(x, keypoint, W_all, b_all):
    """Host-side sharding/layout. Returns per-core in_maps."""
    xT = np.ascontiguousarray(x.T)            # (F, BS)
    kp7 = keypoint[:, KEY_GATHER, :].astype(np.float32) * np.float32(7.0)
    kpg = np.concatenate([kp7[:, :, 0], kp7[:, :, 1]], axis=1)  # (BS, 36)
    kpg = np.ascontiguousarray(kpg)
    consts = _consts_np()
    in_maps = []
    for core in range(N_CORES):
        b0 = core * BPC
        sh = xT[:, b0:b0 + BPC]               # (F, 512)
        # (N_PASS*F, Bp): pass p = cols p*Bp..(p+1)*Bp
        xs = np.concatenate(
            [np.ascontiguousarray(sh[:, p * Bp:(p + 1) * Bp])
             for p in range(N_PASS)], axis=0)
        in_maps.append({
            "xT": xs,
            "w": W_all,
            "b": b_all,
            "kp": np.ascontiguousarray(kpg[b0:b0 + BPC]),
            "consts": consts,
        })
    return in_maps


def _split_output(full):
    """(4096, 441) -> tuple of 5 group tensors."""
    outs = []
    off = 0
    for g in (2, 2, 2, 2, 1):
        outs.append(full[:, off:off + g * 49].reshape(BS, g, 1, WD, WD))
        off += g * 49
    return tuple(outs)


def kernel(**inputs):
    from concourse.bass_utils import run_bass_kernel_spmd

    W_all = np.concatenate(
        [inputs["W_head"], inputs["W_arm"], inputs["W_upper"],
         inputs["W_lower"], inputs["W_foot"]], axis=1).astype(np.float32)
    W_all = np.ascontiguousarray(W_all)
    b_all = np.concatenate(
        [inputs["b_head"], inputs["b_arm"], inputs["b_upper"],
         inputs["b_lower"], inputs["b_foot"]]).astype(np.float32)
    in_maps = _prep_inputs(np.asarray(inputs["x"], np.float32),
                           np.asarray(inputs["keypoint"], np.float32),
                           W_all, b_all)
    if "nc" not in _cached:
        _cached["nc"] = build_program(1)
    res = run_bass_kernel_spmd(_cached["nc"], in_maps,
                               core_ids=list(range(N_CORES)))
    full = np.concatenate([res.results[c]["out"] for c in range(N_CORES)],
                          axis=0)
    return _split_output(full)


# revision 6
# speedup vs baseline: 3.1585x; 2.8897x over previous
"""Trainium2 Bass kernel for nn_KeyHeatModule (8-core data parallel).

Strategy: shard batch (4096) across 8 cores (512 each). Host pre-transposes x
so each core streams contiguous [feat, batch] tiles; one fused weight matrix
W_all (8330 x 126) is replicated. Per core: fp32 matmul accumulates
vec.T = W_all.T @ x_shard.T in PSUM per 128-sample pass, PE-transposes back to
batch-major, then DVE/ACT compute softmax/exp, keypoint range masks, segmented
cumsum heatmap math, per-group max normalization. Output (512, 441) per core is
gathered and split into the 5 group tensors on host.
"""
import sys
for _p in ("/opt/trn_rl_repo", "/root/.axon_site/_ro/trn_rl_repo"):
    if _p not in sys.path:
        sys.path.insert(0, _p)

import numpy as np

N_CORES = 8
BS = 4096
F = 8330          # in features
M = 126           # total linear out cols (28*4 + 14)
WD = 7
SCALE = 0.3
BPC = BS // N_CORES          # 512 samples per core
import os as _os
N_PASS = int(_os.environ.get("KH_N_PASS", "4"))   # batch passes per core
Bp = BPC // N_PASS
NCH = 9                      # total channels across groups (2,2,2,2,1)
KEY_GATHER = [0, 1, 3, 4, 7, 8, 9, 10, 5, 6, 11, 12, 11, 12, 13, 14, 15, 16]
GROUP_SIZES = [4, 4, 4, 4, 2]    # gathered keypoint count per group
KT_FULL = F // 128           # 65 full k-tiles
K_TAIL = F - KT_FULL * 128   # 10
KBW = 8                      # W k-tiles per DMA batch
KBX = int(_os.environ.get("KH_KBX", "16"))   # x k-tiles per DMA batch
XBUFS = int(_os.environ.get("KH_XBUFS", "3"))
ABLATE = _os.environ.get("KH_ABLATE", "full")  # full | mm | dma
MM_MODE = _os.environ.get("KH_MM", "bf16")     # bf16 | fp32

_cached = {}


def _consts_np():
    """Host-built constant tile (128, 205)."""
    c = np.zeros((128, 205), np.float32)
    seg = np.ones(63, np.float32)
    seg[0::7] = 0.0                       # reset at w==0 of each 7-block
    c[:, 0:63] = seg[None, :]
    c[:, 63:70] = (np.arange(7) + 2).astype(np.float32)[None, :]   # iota2
    cap = np.arange(7).astype(np.float32)
    cap[6] = 1e9                          # w=6 never kept by (w < hi<=6)
    c[:, 70:77] = cap[None, :]
    c[:, 77:205] = np.eye(128, dtype=np.float32)
    return c


def build_program(n_reps=1):
    import concourse.bacc as bacc
    import concourse.tile as tile
    from concourse import mybir

    f32 = mybir.dt.float32
    A = mybir.AluOpType
    ACT = mybir.ActivationFunctionType
    AX = mybir.AxisListType

    bf16 = mybir.dt.bfloat16
    mdt = f32 if MM_MODE == "fp32" else bf16
    nhalf = 1 if MM_MODE == "fp32" else 2
    nc = bacc.Bacc(num_devices=N_CORES)
    xT_d = nc.dram_tensor("xT", (nhalf, N_PASS * F, Bp), mdt,
                          kind="ExternalInput")
    w_d = nc.dram_tensor("w", (nhalf, F, M), mdt, kind="ExternalInput")
    b_d = nc.dram_tensor("b", (M,), f32, kind="ExternalInput")
    kp_d = nc.dram_tensor("kp", (BPC, 36), f32, kind="ExternalInput")
    c_d = nc.dram_tensor("consts", (128, 205), f32, kind="ExternalInput")
    out_d = nc.dram_tensor("out", (BPC, 441), f32, kind="ExternalOutput")

    def bc(ap, shape):
        return ap.to_broadcast(tuple(shape))

    with tile.TileContext(nc) as tc:
        import contextlib
        with contextlib.ExitStack() as ctx:
            singles = ctx.enter_context(tc.tile_pool(name="singles", bufs=1))
            xpool = ctx.enter_context(tc.tile_pool(name="xpool", bufs=XBUFS))
            pspool = ctx.enter_context(tc.tile_pool(name="ps", bufs=2, space="PSUM"))
            work = ctx.enter_context(tc.tile_pool(name="work", bufs=2))

            # ---- constants / weights (once) ----
            CT = singles.tile([128, 205], f32)
            nc.sync.dma_start(CT[:], c_d[:, :])
            SEG = CT[:, 0:63]
            IOTA2 = CT[:, 63:70]
            IOCAP = CT[:, 70:77]
            IDENT = CT[:, 77:205]

            BT = singles.tile([128, 1], f32)
            nc.sync.dma_start(BT[:126, :], b_d[:].rearrange("(p o) -> p o", o=1))

            wtiles = {}          # (half, ktile) -> AP [128 or 10, 126]
            for h in range(nhalf):
                for i, k0 in enumerate(range(0, KT_FULL, KBW)):
                    nk = min(KBW, KT_FULL - k0)
                    wt = singles.tile([128, nk, M], mdt, tag=f"wb{h}_{i}")
                    nc.sync.dma_start(
                        wt[:],
                        w_d[h, k0 * 128:(k0 + nk) * 128, :].rearrange(
                            "(s p) m -> p s m", p=128))
                    for s in range(nk):
                        wtiles[(h, k0 + s)] = wt[:, s, :]
                wtl = singles.tile([128, M], mdt, tag=f"wtail{h}")
                nc.sync.dma_start(wtl[:K_TAIL, :],
                                  w_d[h, KT_FULL * 128:F, :])
                wtiles[(h, KT_FULL)] = wtl[:K_TAIL, :]

            for rep in range(n_reps):
                for p in range(N_PASS):
                    # ---- x DMA (contiguous ~1MB batches) + matmul ----
                    xbs = []
                    for k0 in range(0, KT_FULL, KBX):
                        nk = min(KBX, KT_FULL - k0)
                        xhalves = []
                        for h in range(nhalf):
                            xb = xpool.tile([128, nk, Bp], mdt, tag=f"xb{h}")
                            r0 = p * F + k0 * 128
                            nc.sync.dma_start(
                                xb[:, :nk, :],
                                xT_d[h, r0:r0 + nk * 128, :].rearrange(
                                    "(s q) n -> q s n", q=128))
                            xhalves.append(xb)
                        xbs.append((k0, nk, xhalves))
                    xtails = []
                    for h in range(nhalf):
                        xtl = xpool.tile([128, Bp], mdt, tag=f"xtail{h}")
                        nc.sync.dma_start(
                            xtl[:K_TAIL, :],
                            xT_d[h, p * F + KT_FULL * 128:p * F + F, :])
                        xtails.append(xtl)

                    if ABLATE == "dma":
                        tch = work.tile([128, Bp], f32, tag="touch")
                        nc.vector.tensor_copy(tch[:], xbs[-1][2][0][:, 0, :].bitcast(f32) if nhalf==2 else xbs[-1][2][0][:, 0, :])
                        nc.sync.dma_start(
                            out_d[p * Bp:p * Bp + 128, 0:Bp // 4],
                            tch[:, 0:Bp // 4])
                        continue
                    ps = pspool.tile([M, Bp], f32, tag="ps")
                    first = True
                    for (k0, nk, xhalves) in xbs:
                        for s in range(nk):
                            k = k0 + s
                            if nhalf == 1:
                                nc.tensor.matmul(
                                    ps[:], wtiles[(0, k)], xhalves[0][:, s, :],
                                    start=first, stop=False)
                            else:
                                # Wh.T xh + Wh.T xl + Wl.T xh
                                nc.tensor.matmul(
                                    ps[:], wtiles[(0, k)], xhalves[0][:, s, :],
                                    start=first, stop=False)
                                nc.tensor.matmul(
                                    ps[:], wtiles[(0, k)], xhalves[1][:, s, :],
                                    start=False, stop=False)
                                nc.tensor.matmul(
                                    ps[:], wtiles[(1, k)], xhalves[0][:, s, :],
                                    start=False, stop=False)
                            first = False
                    kT = KT_FULL
                    if nhalf == 1:
                        nc.tensor.matmul(ps[:], wtiles[(0, kT)],
                                         xtails[0][:K_TAIL, :],
                                         start=False, stop=True)
                    else:
                        nc.tensor.matmul(ps[:], wtiles[(0, kT)],
                                         xtails[0][:K_TAIL, :],
                                         start=False, stop=False)
                        nc.tensor.matmul(ps[:], wtiles[(0, kT)],
                                         xtails[1][:K_TAIL, :],
                                         start=False, stop=False)
                        nc.tensor.matmul(ps[:], wtiles[(1, kT)],
                                         xtails[0][:K_TAIL, :],
                                         start=False, stop=True)

                    for cc in range(Bp // 128):
                        c = p * (Bp // 128) + cc      # global chunk 0..3
                        # ---- bias add + transpose to batch-major ----
                        VEC = work.tile([128, 128], f32, tag="vec")
                        nc.scalar.activation(
                            VEC[:M, :], ps[:, cc * 128:(cc + 1) * 128],
                            ACT.Identity, bias=BT[:M, :], scale=1.0)
                        TP = pspool.tile([128, M], f32, tag="tp")
                        nc.tensor.transpose(TP[:], VEC[:M, :], IDENT[:M, :M])
                        VB = work.tile([128, M], f32, tag="vb")
                        nc.scalar.copy(VB[:], TP[:])
                        if ABLATE == "mm":
                            nc.sync.dma_start(
                                out_d[c * 128:(c + 1) * 128, 0:M], VB[:])
                            continue

                        # ---- keypoint tile + group ranges ----
                        KPT = work.tile([128, 36], f32, tag="kpt")
                        nc.sync.dma_start(KPT[:], kp_d[c * 128:(c + 1) * 128, :])
                        MMs = work.tile([128, 20], f32, tag="mms")
                        for j, (off, op) in enumerate(
                                [(0, A.min), (0, A.max), (18, A.min), (18, A.max)]):
                            nc.vector.tensor_reduce(
                                MMs[:, j * 5:j * 5 + 4],
                                KPT[:, off:off + 16].rearrange(
                                    "p (g k) -> p g k", k=4),
                                AX.X, op)
                            nc.vector.tensor_reduce(
                                MMs[:, j * 5 + 4:j * 5 + 5],
                                KPT[:, off + 16:off + 18], AX.X, op)
                        # channel-level stats CS [128, 36]
                        CS = work.tile([128, 36], f32, tag="cs")
                        for j in range(4):
                            nc.gpsimd.tensor_copy(
                                CS[:, j * 9:j * 9 + 8].rearrange(
                                    "p (g r) -> p g r", r=2),
                                bc(MMs[:, j * 5:j * 5 + 4].rearrange(
                                    "p (g o) -> p g o", o=1), (128, 4, 2)))
                            nc.gpsimd.tensor_copy(
                                CS[:, j * 9 + 8:j * 9 + 9],
                                MMs[:, j * 5 + 4:j * 5 + 5])

                        # ---- keeps [128, 63] each ----
                        KEEP = work.tile([128, 4, NCH, WD], f32, tag="keep")
                        i2b = bc(IOTA2.rearrange("p (o w) -> p o w", o=1),
                                 (128, NCH, WD))
                        icb = bc(IOCAP.rearrange("p (o w) -> p o w", o=1),
                                 (128, NCH, WD))
                        for j, (cso, one_is_iota) in enumerate(
                                [(0, True), (9, False), (18, True), (27, False)]):
                            csb = bc(CS[:, cso:cso + 9].rearrange(
                                "p (c o) -> p c o", o=1), (128, NCH, WD))
                            if one_is_iota:   # keep1 = iota2 > fmin
                                nc.vector.tensor_tensor(
                                    KEEP[:, j, :, :], i2b, csb, A.is_gt)
                            else:             # keep2 = fmax >= iota_cap
                                nc.vector.tensor_tensor(
                                    KEEP[:, j, :, :], csb, icb, A.is_ge)

                        # ---- softmax-pair + exp ----
                        VB3 = VB.rearrange("p (c s w) -> p c s w", s=2, w=WD)
                        D = work.tile([128, NCH, WD], f32, tag="d")
                        nc.vector.tensor_tensor(
                            D[:], VB3[:, :, 0, :], VB3[:, :, 1, :], A.subtract)
                        V0 = work.tile([128, NCH, WD], f32, tag="v0")
                        V1 = work.tile([128, NCH, WD], f32, tag="v1")
                        S0 = work.tile([128, NCH, WD], f32, tag="s0")
                        S1 = work.tile([128, NCH, WD], f32, tag="s1")
                        nc.scalar.activation(S0[:], D[:], ACT.Sigmoid)
                        nc.scalar.activation(S1[:], D[:], ACT.Sigmoid, scale=-1.0)
                        nc.scalar.activation(V0[:], S0[:], ACT.Exp)
                        nc.scalar.activation(V1[:], S1[:], ACT.Exp)

                        # ---- masked segmented cumsum -> thresholds ----
                        AM = work.tile([128, 4, NCH, WD], f32, tag="am")
                        for j, V in enumerate([V0, V0, V1, V1]):
                            nc.vector.tensor_mul(AM[:, j, :, :], V[:],
                                                 KEEP[:, j, :, :])
                        C = work.tile([128, 4, NCH, WD], f32, tag="cum")
                        for j in range(4):
                            nc.vector.tensor_tensor_scan(
                                C[:, j, :, :].rearrange("p c w -> p (c w)"),
                                SEG,
                                AM[:, j, :, :].rearrange("p c w -> p (c w)"),
                                0.0, A.mult, A.add)
                        R = work.tile([128, 4, NCH], f32, tag="recip")
                        for j in range(4):
                            nc.vector.reciprocal(
                                R[:, j, :].unsqueeze(2),
                                C[:, j, :, 6:7])
                        TH = work.tile([128, 4, NCH, WD], f32, tag="th")
                        T2 = work.tile([128, 2, NCH, WD], f32, tag="t2")
                        for j in range(4):
                            rb = bc(R[:, j, :].unsqueeze(2),
                                    (128, NCH, WD))
                            if j % 2 == 0:    # c1 = cum/S ; thresh
                                nc.vector.tensor_mul(TH[:, j, :, :],
                                                     C[:, j, :, :], rb)
                                nc.vector.scalar_tensor_tensor(
                                    TH[:, j, :, :], TH[:, j, :, :], SCALE,
                                    TH[:, j, :, :], A.is_ge, A.mult)
                            else:             # c2 = 1 - cum/S ; thresh
                                t2 = T2[:, j // 2, :, :]
                                nc.vector.tensor_mul(t2, C[:, j, :, :], rb)
                                nc.vector.tensor_scalar(
                                    t2, t2, -1.0, 1.0, A.mult, A.add)
                                nc.vector.scalar_tensor_tensor(
                                    TH[:, j, :, :], t2, SCALE, t2,
                                    A.is_ge, A.mult)
                        XM = work.tile([128, NCH, WD], f32, tag="xm")
                        YM = work.tile([128, NCH, WD], f32, tag="ym")
                        nc.vector.tensor_mul(XM[:], TH[:, 0, :, :], TH[:, 1, :, :])
                        nc.vector.tensor_mul(YM[:], TH[:, 2, :, :], TH[:, 3, :, :])

                        # ---- outer product + per-group max normalize ----
                        MO = work.tile([128, NCH, WD, WD], f32, tag="mo")
                        nc.vector.tensor_mul(
                            MO[:],
                            bc(YM[:].unsqueeze(3),
                               (128, NCH, WD, WD)),
                            bc(XM[:].unsqueeze(2),
                               (128, NCH, WD, WD)))
                        CHM = work.tile([128, NCH], f32, tag="chm")
                        nc.vector.tensor_reduce(
                            CHM[:], MO.rearrange("p c y x -> p c (y x)"),
                            AX.X, A.max)
                        GM = work.tile([128, 8], f32, tag="gm")
                        CHM3 = CHM[:, 0:8].rearrange("p (g r) -> p g r", r=2)
                        nc.vector.tensor_tensor(
                            GM[:, 0:4].rearrange("p (g o) -> p g o", o=1),
                            CHM3[:, :, 0:1], CHM3[:, :, 1:2], A.max)
                        nc.gpsimd.tensor_copy(GM[:, 4:5], CHM[:, 8:9])
                        nc.vector.tensor_scalar(GM[:, 5:6], GM[:, 4:5], 1e-7,
                                                None, A.add)
                        RM = work.tile([128, 9], f32, tag="rm")
                        nc.vector.tensor_scalar(GM[:, 0:4], GM[:, 0:4], 1e-7,
                                                None, A.add)
                        nc.vector.reciprocal(RM[:, 0:4], GM[:, 0:4])
                        nc.vector.reciprocal(RM[:, 4:5], GM[:, 5:6])
                        RC = work.tile([128, NCH], f32, tag="rc")
                        nc.gpsimd.tensor_copy(
                            RC[:, 0:8].rearrange("p (g r) -> p g r", r=2),
                            bc(RM[:, 0:4].rearrange("p (g o) -> p g o", o=1),
                               (128, 4, 2)))
                        nc.gpsimd.tensor_copy(RC[:, 8:9], RM[:, 4:5])
                        MN = work.tile([128, NCH, WD * WD], f32, tag="mn")
                        nc.vector.tensor_mul(
                            MN[:], MO.rearrange("p c y x -> p c (y x)"),
                            bc(RC[:].unsqueeze(2), (128, NCH, WD * WD)))
                        nc.sync.dma_start(out_d[c * 128:(c + 1) * 128, :],
                                          MN.rearrange("p c q -> p (c q)"))
    nc.compile()
    return nc


def _split_hi_lo(a):
    import ml_dtypes
    hi = a.astype(ml_dtypes.bfloat16)
    lo = (a - hi.astype(np.float32)).astype(ml_dtypes.bfloat16)
    return hi, lo


def _prep_inputs(x, keypoint, W_all, b_all):
    """Host-side sharding/layout. Returns per-core in_maps."""
    import os
    mm_mode = os.environ.get("KH_MM", "bf16")
    xT = np.ascontiguousarray(x.T)            # (F, BS)
    kp7 = keypoint[:, KEY_GATHER, :].astype(np.float32) * np.float32(7.0)
    kpg = np.concatenate([kp7[:, :, 0], kp7[:, :, 1]], axis=1)  # (BS, 36)
    kpg = np.ascontiguousarray(kpg)
    consts = _consts_np()
    in_maps = []
    if mm_mode == "bf16":
        wh, wl = _split_hi_lo(W_all)
        w_send = np.stack([wh, wl])
    else:
        w_send = W_all[None]
    for core in range(N_CORES):
        b0 = core * BPC
        sh = xT[:, b0:b0 + BPC]               # (F, 512)
        # (N_PASS*F, Bp): pass p = cols p*Bp..(p+1)*Bp
        xs = np.concatenate(
            [np.ascontiguousarray(sh[:, p * Bp:(p + 1) * Bp])
             for p in range(N_PASS)], axis=0)
        if mm_mode == "bf16":
            xh, xl = _split_hi_lo(xs)
            x_send = np.stack([xh, xl])
        else:
            x_send = xs[None]
        in_maps.append({
            "xT": x_send,
            "w": w_send,
            "b": b_all,
            "kp": np.ascontiguousarray(kpg[b0:b0 + BPC]),
            "consts": consts,
        })
    return in_maps


def _split_output(full):
    """(4096, 441) -> tuple of 5 group tensors."""
    outs = []
    off = 0
    for g in (2, 2, 2, 2, 1):
        outs.append(full[:, off:off + g * 49].reshape(BS, g, 1, WD, WD))
        off += g * 49
    return tuple(outs)


def kernel(**inputs):
    from concourse.bass_utils import run_bass_kernel_spmd

    W_all = np.concatenate(
        [inputs["W_head"], inputs["W_arm"], inputs["W_upper"],
         inputs["W_lower"], inputs["W_foot"]], axis=1).astype(np.float32)
    W_all = np.ascontiguousarray(W_all)
    b_all = np.concatenate(
        [inputs["b_head"], inputs["b_arm"], inputs["b_upper"],
         inputs["b_lower"], inputs["b_foot"]]).astype(np.float32)
    in_maps = _prep_inputs(np.asarray(inputs["x"], np.float32),
                           np.asarray(inputs["keypoint"], np.float32),
                           W_all, b_all)
    if "nc" not in _cached:
        _cached["nc"] = build_program(1)
    res = run_bass_kernel_spmd(_cached["nc"], in_maps,
                               core_ids=list(range(N_CORES)))
    full = np.concatenate([res.results[c]["out"] for c in range(N_CORES)],
                          axis=0)
    return _split_output(full)


# revision 7
# speedup vs baseline: 8.1226x; 2.5716x over previous
"""Trainium2 Bass kernel for nn_KeyHeatModule (8-core data parallel).

Strategy: shard batch (4096) across 8 cores (512 each). Host pre-transposes x
so each core streams contiguous [feat, batch] tiles; one fused weight matrix
W_all (8330 x 126) is replicated. Per core: fp32 matmul accumulates
vec.T = W_all.T @ x_shard.T in PSUM per 128-sample pass, PE-transposes back to
batch-major, then DVE/ACT compute softmax/exp, keypoint range masks, segmented
cumsum heatmap math, per-group max normalization. Output (512, 441) per core is
gathered and split into the 5 group tensors on host.
"""
import sys
for _p in ("/opt/trn_rl_repo", "/root/.axon_site/_ro/trn_rl_repo"):
    if _p not in sys.path:
        sys.path.insert(0, _p)

import numpy as np

N_CORES = 8
BS = 4096
F = 8330          # in features
M = 126           # total linear out cols (28*4 + 14)
WD = 7
SCALE = 0.3
BPC = BS // N_CORES          # 512 samples per core
import os as _os
N_PASS = int(_os.environ.get("KH_N_PASS", "4"))   # batch passes per core
Bp = BPC // N_PASS
NCH = 9                      # total channels across groups (2,2,2,2,1)
KEY_GATHER = [0, 1, 3, 4, 7, 8, 9, 10, 5, 6, 11, 12, 11, 12, 13, 14, 15, 16]
GROUP_SIZES = [4, 4, 4, 4, 2]    # gathered keypoint count per group
KT_FULL = F // 128           # 65 full k-tiles
K_TAIL = F - KT_FULL * 128   # 10
KBW = 8                      # W k-tiles per DMA batch
KBX = int(_os.environ.get("KH_KBX", "16"))   # x k-tiles per DMA batch
XBUFS = int(_os.environ.get("KH_XBUFS", "3"))
ABLATE = _os.environ.get("KH_ABLATE", "full")  # full | mm | dma
MM_MODE = _os.environ.get("KH_MM", "bf16")     # bf16 | fp32

_cached = {}


def _consts_np():
    """Host-built constant tile (128, 205)."""
    c = np.zeros((128, 205), np.float32)
    seg = np.ones(63, np.float32)
    seg[0::7] = 0.0                       # reset at w==0 of each 7-block
    c[:, 0:63] = seg[None, :]
    c[:, 63:70] = (np.arange(7) + 2).astype(np.float32)[None, :]   # iota2
    cap = np.arange(7).astype(np.float32)
    cap[6] = 1e9                          # w=6 never kept by (w < hi<=6)
    c[:, 70:77] = cap[None, :]
    c[:, 77:205] = np.eye(128, dtype=np.float32)
    return c


def build_program(n_reps=1):
    import concourse.bacc as bacc
    import concourse.tile as tile
    from concourse import mybir

    f32 = mybir.dt.float32
    A = mybir.AluOpType
    ACT = mybir.ActivationFunctionType
    AX = mybir.AxisListType

    bf16 = mybir.dt.bfloat16
    mdt = f32 if MM_MODE == "fp32" else bf16
    nhalf = 1 if MM_MODE == "fp32" else 2
    nc = bacc.Bacc(num_devices=N_CORES)
    xT_d = nc.dram_tensor("xT", (nhalf, N_PASS * F, Bp), mdt,
                          kind="ExternalInput")
    w_d = nc.dram_tensor("w", (nhalf, F, M), mdt, kind="ExternalInput")
    b_d = nc.dram_tensor("b", (M,), f32, kind="ExternalInput")
    kp_d = nc.dram_tensor("kp", (BPC, 36), f32, kind="ExternalInput")
    c_d = nc.dram_tensor("consts", (128, 205), f32, kind="ExternalInput")
    out_d = nc.dram_tensor("out", (BPC, 441), f32, kind="ExternalOutput")

    def bc(ap, shape):
        return ap.to_broadcast(tuple(shape))

    with tile.TileContext(nc) as tc:
        import contextlib
        with contextlib.ExitStack() as ctx:
            singles = ctx.enter_context(tc.tile_pool(name="singles", bufs=1))
            xpool = ctx.enter_context(tc.tile_pool(name="xpool", bufs=XBUFS))
            pspool = ctx.enter_context(tc.tile_pool(name="ps", bufs=2, space="PSUM"))
            work = ctx.enter_context(tc.tile_pool(name="work", bufs=2))

            # ---- constants / weights (once) ----
            CT = singles.tile([128, 205], f32)
            nc.sync.dma_start(CT[:], c_d[:, :])
            SEG = CT[:, 0:63]
            IOTA2 = CT[:, 63:70]
            IOCAP = CT[:, 70:77]
            IDENT = CT[:, 77:205]

            BT = singles.tile([128, 1], f32)
            nc.sync.dma_start(BT[:126, :], b_d[:].rearrange("(p o) -> p o", o=1))

            wtiles = {}          # (half, ktile) -> AP [128 or 10, 126]
            for h in range(nhalf):
                for i, k0 in enumerate(range(0, KT_FULL, KBW)):
                    nk = min(KBW, KT_FULL - k0)
                    wt = singles.tile([128, nk, M], mdt, tag=f"wb{h}_{i}")
                    nc.gpsimd.dma_start(
                        wt[:],
                        w_d[h, k0 * 128:(k0 + nk) * 128, :].rearrange(
                            "(s p) m -> p s m", p=128))
                    for s in range(nk):
                        wtiles[(h, k0 + s)] = wt[:, s, :]
                wtl = singles.tile([128, M], mdt, tag=f"wtail{h}")
                nc.sync.dma_start(wtl[:K_TAIL, :],
                                  w_d[h, KT_FULL * 128:F, :])
                wtiles[(h, KT_FULL)] = wtl[:K_TAIL, :]

            for rep in range(n_reps):
                for p in range(N_PASS):
                    # ---- x DMA (contiguous ~1MB batches) + matmul ----
                    xbs = []
                    for k0 in range(0, KT_FULL, KBX):
                        nk = min(KBX, KT_FULL - k0)
                        xhalves = []
                        for h in range(nhalf):
                            xb = xpool.tile([128, nk, Bp], mdt, tag=f"xb{h}")
                            r0 = p * F + k0 * 128
                            eng = nc.sync if h == 0 else nc.scalar
                            eng.dma_start(
                                xb[:, :nk, :],
                                xT_d[h, r0:r0 + nk * 128, :].rearrange(
                                    "(s q) n -> q s n", q=128))
                            xhalves.append(xb)
                        xbs.append((k0, nk, xhalves))
                    xtails = []
                    for h in range(nhalf):
                        xtl = xpool.tile([128, Bp], mdt, tag=f"xtail{h}")
                        nc.sync.dma_start(
                            xtl[:K_TAIL, :],
                            xT_d[h, p * F + KT_FULL * 128:p * F + F, :])
                        xtails.append(xtl)

                    if ABLATE == "dma":
                        tch = work.tile([128, Bp], f32, tag="touch")
                        nc.vector.tensor_copy(tch[:], xbs[-1][2][0][:, 0, :].bitcast(f32) if nhalf==2 else xbs[-1][2][0][:, 0, :])
                        nc.sync.dma_start(
                            out_d[p * Bp:p * Bp + 128, 0:Bp // 4],
                            tch[:, 0:Bp // 4])
                        continue
                    ps = pspool.tile([M, Bp], f32, tag="ps")
                    first = True
                    for (k0, nk, xhalves) in xbs:
                        for s in range(nk):
                            k = k0 + s
                            if nhalf == 1:
                                nc.tensor.matmul(
                                    ps[:], wtiles[(0, k)], xhalves[0][:, s, :],
                                    start=first, stop=False)
                            else:
                                # Wh.T xh + Wh.T xl + Wl.T xh
                                nc.tensor.matmul(
                                    ps[:], wtiles[(0, k)], xhalves[0][:, s, :],
                                    start=first, stop=False)
                                nc.tensor.matmul(
                                    ps[:], wtiles[(0, k)], xhalves[1][:, s, :],
                                    start=False, stop=False)
                                nc.tensor.matmul(
                                    ps[:], wtiles[(1, k)], xhalves[0][:, s, :],
                                    start=False, stop=False)
                            first = False
                    kT = KT_FULL
                    if nhalf == 1:
                        nc.tensor.matmul(ps[:], wtiles[(0, kT)],
                                         xtails[0][:K_TAIL, :],
                                         start=False, stop=True)
                    else:
                        nc.tensor.matmul(ps[:], wtiles[(0, kT)],
                                         xtails[0][:K_TAIL, :],
                                         start=False, stop=False)
                        nc.tensor.matmul(ps[:], wtiles[(0, kT)],
                                         xtails[1][:K_TAIL, :],
                                         start=False, stop=False)
                        nc.tensor.matmul(ps[:], wtiles[(1, kT)],
                                         xtails[0][:K_TAIL, :],
                                         start=False, stop=True)

                    for cc in range(Bp // 128):
                        c = p * (Bp // 128) + cc      # global chunk 0..3
                        # ---- bias add + transpose to batch-major ----
                        VEC = work.tile([128, 128], f32, tag="vec")
                        nc.scalar.activation(
                            VEC[:M, :], ps[:, cc * 128:(cc + 1) * 128],
                            ACT.Identity, bias=BT[:M, :], scale=1.0)
                        TP = pspool.tile([128, M], f32, tag="tp")
                        nc.tensor.transpose(TP[:], VEC[:M, :], IDENT[:M, :M])
                        VB = work.tile([128, M], f32, tag="vb")
                        nc.scalar.copy(VB[:], TP[:])
                        if ABLATE == "mm":
                            nc.sync.dma_start(
                                out_d[c * 128:(c + 1) * 128, 0:M], VB[:])
                            continue

                        # ---- keypoint tile + group ranges ----
                        KPT = work.tile([128, 36], f32, tag="kpt")
                        nc.sync.dma_start(KPT[:], kp_d[c * 128:(c + 1) * 128, :])
                        MMs = work.tile([128, 20], f32, tag="mms")
                        for j, (off, op) in enumerate(
                                [(0, A.min), (0, A.max), (18, A.min), (18, A.max)]):
                            nc.vector.tensor_reduce(
                                MMs[:, j * 5:j * 5 + 4],
                                KPT[:, off:off + 16].rearrange(
                                    "p (g k) -> p g k", k=4),
                                AX.X, op)
                            nc.vector.tensor_reduce(
                                MMs[:, j * 5 + 4:j * 5 + 5],
                                KPT[:, off + 16:off + 18], AX.X, op)
                        # channel-level stats CS [128, 36]
                        CS = work.tile([128, 36], f32, tag="cs")
                        for j in range(4):
                            nc.gpsimd.tensor_copy(
                                CS[:, j * 9:j * 9 + 8].rearrange(
                                    "p (g r) -> p g r", r=2),
                                bc(MMs[:, j * 5:j * 5 + 4].rearrange(
                                    "p (g o) -> p g o", o=1), (128, 4, 2)))
                            nc.gpsimd.tensor_copy(
                                CS[:, j * 9 + 8:j * 9 + 9],
                                MMs[:, j * 5 + 4:j * 5 + 5])

                        # ---- keeps [128, 63] each ----
                        KEEP = work.tile([128, 4, NCH, WD], f32, tag="keep")
                        i2b = bc(IOTA2.rearrange("p (o w) -> p o w", o=1),
                                 (128, NCH, WD))
                        icb = bc(IOCAP.rearrange("p (o w) -> p o w", o=1),
                                 (128, NCH, WD))
                        for j, (cso, one_is_iota) in enumerate(
                                [(0, True), (9, False), (18, True), (27, False)]):
                            csb = bc(CS[:, cso:cso + 9].rearrange(
                                "p (c o) -> p c o", o=1), (128, NCH, WD))
                            if one_is_iota:   # keep1 = iota2 > fmin
                                nc.vector.tensor_tensor(
                                    KEEP[:, j, :, :], i2b, csb, A.is_gt)
                            else:             # keep2 = fmax >= iota_cap
                                nc.vector.tensor_tensor(
                                    KEEP[:, j, :, :], csb, icb, A.is_ge)

                        # ---- softmax-pair + exp ----
                        VB3 = VB.rearrange("p (c s w) -> p c s w", s=2, w=WD)
                        D = work.tile([128, NCH, WD], f32, tag="d")
                        nc.vector.tensor_tensor(
                            D[:], VB3[:, :, 0, :], VB3[:, :, 1, :], A.subtract)
                        V0 = work.tile([128, NCH, WD], f32, tag="v0")
                        V1 = work.tile([128, NCH, WD], f32, tag="v1")
                        S0 = work.tile([128, NCH, WD], f32, tag="s0")
                        S1 = work.tile([128, NCH, WD], f32, tag="s1")
                        nc.scalar.activation(S0[:], D[:], ACT.Sigmoid)
                        nc.scalar.activation(S1[:], D[:], ACT.Sigmoid, scale=-1.0)
                        nc.scalar.activation(V0[:], S0[:], ACT.Exp)
                        nc.scalar.activation(V1[:], S1[:], ACT.Exp)

                        # ---- masked segmented cumsum -> thresholds ----
                        AM = work.tile([128, 4, NCH, WD], f32, tag="am")
                        for j, V in enumerate([V0, V0, V1, V1]):
                            nc.vector.tensor_mul(AM[:, j, :, :], V[:],
                                                 KEEP[:, j, :, :])
                        C = work.tile([128, 4, NCH, WD], f32, tag="cum")
                        for j in range(4):
                            nc.vector.tensor_tensor_scan(
                                C[:, j, :, :].rearrange("p c w -> p (c w)"),
                                SEG,
                                AM[:, j, :, :].rearrange("p c w -> p (c w)"),
                                0.0, A.mult, A.add)
                        R = work.tile([128, 4, NCH], f32, tag="recip")
                        for j in range(4):
                            nc.vector.reciprocal(
                                R[:, j, :].unsqueeze(2),
                                C[:, j, :, 6:7])
                        TH = work.tile([128, 4, NCH, WD], f32, tag="th")
                        T2 = work.tile([128, 2, NCH, WD], f32, tag="t2")
                        for j in range(4):
                            rb = bc(R[:, j, :].unsqueeze(2),
                                    (128, NCH, WD))
                            if j % 2 == 0:    # c1 = cum/S ; thresh
                                nc.vector.tensor_mul(TH[:, j, :, :],
                                                     C[:, j, :, :], rb)
                                nc.vector.scalar_tensor_tensor(
                                    TH[:, j, :, :], TH[:, j, :, :], SCALE,
                                    TH[:, j, :, :], A.is_ge, A.mult)
                            else:             # c2 = 1 - cum/S ; thresh
                                t2 = T2[:, j // 2, :, :]
                                nc.vector.tensor_mul(t2, C[:, j, :, :], rb)
                                nc.vector.tensor_scalar(
                                    t2, t2, -1.0, 1.0, A.mult, A.add)
                                nc.vector.scalar_tensor_tensor(
                                    TH[:, j, :, :], t2, SCALE, t2,
                                    A.is_ge, A.mult)
                        XM = work.tile([128, NCH, WD], f32, tag="xm")
                        YM = work.tile([128, NCH, WD], f32, tag="ym")
                        nc.vector.tensor_mul(XM[:], TH[:, 0, :, :], TH[:, 1, :, :])
                        nc.vector.tensor_mul(YM[:], TH[:, 2, :, :], TH[:, 3, :, :])

                        # ---- outer product + per-group max normalize ----
                        MO = work.tile([128, NCH, WD, WD], f32, tag="mo")
                        nc.vector.tensor_mul(
                            MO[:],
                            bc(YM[:].unsqueeze(3),
                               (128, NCH, WD, WD)),
                            bc(XM[:].unsqueeze(2),
                               (128, NCH, WD, WD)))
                        CHM = work.tile([128, NCH], f32, tag="chm")
                        nc.vector.tensor_reduce(
                            CHM[:], MO.rearrange("p c y x -> p c (y x)"),
                            AX.X, A.max)
                        GM = work.tile([128, 8], f32, tag="gm")
                        CHM3 = CHM[:, 0:8].rearrange("p (g r) -> p g r", r=2)
                        nc.vector.tensor_tensor(
                            GM[:, 0:4].rearrange("p (g o) -> p g o", o=1),
                            CHM3[:, :, 0:1], CHM3[:, :, 1:2], A.max)
                        nc.gpsimd.tensor_copy(GM[:, 4:5], CHM[:, 8:9])
                        nc.vector.tensor_scalar(GM[:, 5:6], GM[:, 4:5], 1e-7,
                                                None, A.add)
                        RM = work.tile([128, 9], f32, tag="rm")
                        nc.vector.tensor_scalar(GM[:, 0:4], GM[:, 0:4], 1e-7,
                                                None, A.add)
                        nc.vector.reciprocal(RM[:, 0:4], GM[:, 0:4])
                        nc.vector.reciprocal(RM[:, 4:5], GM[:, 5:6])
                        RC = work.tile([128, NCH], f32, tag="rc")
                        nc.gpsimd.tensor_copy(
                            RC[:, 0:8].rearrange("p (g r) -> p g r", r=2),
                            bc(RM[:, 0:4].rearrange("p (g o) -> p g o", o=1),
                               (128, 4, 2)))
                        nc.gpsimd.tensor_copy(RC[:, 8:9], RM[:, 4:5])
                        MN = work.tile([128, NCH, WD * WD], f32, tag="mn")
                        nc.vector.tensor_mul(
                            MN[:], MO.rearrange("p c y x -> p c (y x)"),
                            bc(RC[:].unsqueeze(2), (128, NCH, WD * WD)))
                        nc.sync.dma_start(out_d[c * 128:(c + 1) * 128, :],
                                          MN.rearrange("p c q -> p (c q)"))
    nc.compile()
    return nc


def _split_hi_lo(a):
    import ml_dtypes
    hi = a.astype(ml_dtypes.bfloat16)
    lo = (a - hi.astype(np.float32)).astype(ml_dtypes.bfloat16)
    return hi, lo


def _prep_inputs(x, keypoint, W_all, b_all):
    """Host-side sharding/layout. Returns per-core in_maps."""
    import os
    mm_mode = os.environ.get("KH_MM", "bf16")
    xT = np.ascontiguousarray(x.T)            # (F, BS)
    kp7 = keypoint[:, KEY_GATHER, :].astype(np.float32) * np.float32(7.0)
    kpg = np.concatenate([kp7[:, :, 0], kp7[:, :, 1]], axis=1)  # (BS, 36)
    kpg = np.ascontiguousarray(kpg)
    consts = _consts_np()
    in_maps = []
    if mm_mode == "bf16":
        wh, wl = _split_hi_lo(W_all)
        w_send = np.stack([wh, wl])
    else:
        w_send = W_all[None]
    for core in range(N_CORES):
        b0 = core * BPC
        sh = xT[:, b0:b0 + BPC]               # (F, 512)
        # (N_PASS*F, Bp): pass p = cols p*Bp..(p+1)*Bp
        xs = np.concatenate(
            [np.ascontiguousarray(sh[:, p * Bp:(p + 1) * Bp])
             for p in range(N_PASS)], axis=0)
        if mm_mode == "bf16":
            xh, xl = _split_hi_lo(xs)
            x_send = np.stack([xh, xl])
        else:
            x_send = xs[None]
        in_maps.append({
            "xT": x_send,
            "w": w_send,
            "b": b_all,
            "kp": np.ascontiguousarray(kpg[b0:b0 + BPC]),
            "consts": consts,
        })
    return in_maps


def _split_output(full):
    """(4096, 441) -> tuple of 5 group tensors."""
    outs = []
    off = 0
    for g in (2, 2, 2, 2, 1):
        outs.append(full[:, off:off + g * 49].reshape(BS, g, 1, WD, WD))
        off += g * 49
    return tuple(outs)


def kernel(**inputs):
    from concourse.bass_utils import run_bass_kernel_spmd

    W_all = np.concatenate(
        [inputs["W_head"], inputs["W_arm"], inputs["W_upper"],
         inputs["W_lower"], inputs["W_foot"]], axis=1).astype(np.float32)
    W_all = np.ascontiguousarray(W_all)
    b_all = np.concatenate(
        [inputs["b_head"], inputs["b_arm"], inputs["b_upper"],
         inputs["b_lower"], inputs["b_foot"]]).astype(np.float32)
    in_maps = _prep_inputs(np.asarray(inputs["x"], np.float32),
                           np.asarray(inputs["keypoint"], np.float32),
                           W_all, b_all)
    if "nc" not in _cached:
        _cached["nc"] = build_program(1)
    res = run_bass_kernel_spmd(_cached["nc"], in_maps,
                               core_ids=list(range(N_CORES)))
    full = np.concatenate([res.results[c]["out"] for c in range(N_CORES)],
                          axis=0)
    return _split_output(full)
